# revision 1
# baseline (speedup 1.0000x reference)
"""Trainium2 Bass kernel for GCNN message passing.

out[b] = relu((A @ x[b]) @ W + bias),  A sparse [N, N] from 800k edges.

Sharding (8 NeuronCores): core h owns output rows [h*6272, (h+1)*6272) for
ALL 4 batches. Host interleaves x into xcat[n] = x[:, n, :] (bf16,
[N, 4*128]) so ONE gather descriptor fetches a neighbor's features for all
4 batches at once (Q7 descriptor generation is the bottleneck resource, at
~8ns per gather index).

Device algorithm per core:
  Host pre-sorts the core's ~100k edges by destination row into 25
  row-blocks of 256 rows; within a block edges are split into "low"
  (col < 32768) / "high" groups so gather indices fit in int16, padded to
  uniform L / H tiles of 128 edges (col=0/val=0 padding).
  The scaled one-hot scatter matrices S[e, r] = (r == rl[e]) * val[e]
  ([128, 256] bf16 per edge-tile) are PREBUILT ON HOST and streamed in
  (DMA has headroom).
  Per row-block:
    - two dma_gather ops (bases xcat[0:], xcat[32768:]) fetch
      msgs [128(edge), T, 512] bf16; edge slot k -> partition k%128,
      tile k//128.
    - PE accumulates aggT_b[c, r] += msgs[:, t, b*128:+128].T @ S_t into
      PSUM [128, 4*256] f32 (segment sum via matmul accumulation).
    - aggT -> SBUF bf16, PE applies W (outT_b = W.T @ aggT_b) into a
      second PSUM tile, ACT applies relu(.+bias), batched DMA writes
      outT [4, 128, 6400] f32.
  Host transposes/concatenates the 8 per-core outputs.
"""
import sys

import numpy as np

try:  # concourse (Bass) lives in the trn repo
    import concourse  # noqa: F401
except ImportError:  # pragma: no cover
    sys.path.insert(0, "/opt/trn_rl_repo")

import ml_dtypes

B, N, E, C = 4, 50000, 800000, 128
LAST_RESULTS = None  # BassKernelResults of the most recent kernel() call
P = 128
BR = 256            # rows per block
RB = 25             # row-blocks per core (covers 6400 >= 6272 rows)
RH = 6272           # row stride between cores (8 * 6272 = 50176 >= N)
NCORES = 8
SPLIT = 32768       # low/high column split for int16 gather indices
OUT_DMA_BLKS = 4    # row-blocks per output DMA


def _pack_idx(vals, n_slots):
    """dma_gather int16 index layout: index k at [k % 16, k // 16],
    replicated to 128 partitions; 0-padded. -> [128, n_slots // 16]"""
    buf = np.zeros(n_slots, np.int16)
    buf[:len(vals)] = vals
    tile16 = buf.reshape(n_slots // 16, 16).T
    return np.tile(tile16, (8, 1))


def _preprocess(edge_row, edge_col, edge_vals):
    """Per-core gather-index tables and host-built S matrices.

    Returns (lowidx [8, RB, 128, 8L], highidx [8, RB, 128, 8H],
             smat [8, 128, RB*T*BR] bf16, L, H).
    Edge slot k of a block: partition k%128, tile k//128; slots < L*128
    low-group (col), the rest high-group (col - SPLIT).
    S tile t of block blk lives at smat[:, (blk*T+t)*BR:(blk*T+t+1)*BR].
    """
    per_core = []
    maxlow = maxhigh = 0
    for h in range(NCORES):
        lo, hi = h * RH, min((h + 1) * RH, N)
        m = (edge_row >= lo) & (edge_row < hi)
        r, c, v = edge_row[m] - lo, edge_col[m], edge_vals[m]
        is_high = c >= SPLIT
        order = np.lexsort((is_high, r // BR))
        r, c, v, is_high = r[order], c[order], v[order], is_high[order]
        blocks = []
        for blk in range(RB):
            sel = slice(*np.searchsorted(r // BR, [blk, blk + 1]))
            rb, cb, vb, hb = r[sel], c[sel], v[sel], is_high[sel]
            nlow = int((~hb).sum())
            blocks.append((rb, cb, vb, nlow))
            maxlow = max(maxlow, nlow)
            maxhigh = max(maxhigh, len(rb) - nlow)
        per_core.append(blocks)
    L = (maxlow + P - 1) // P
    H = (maxhigh + P - 1) // P
    T = L + H
    lowidx = np.zeros((NCORES, RB, P, 8 * L), np.int16)
    highidx = np.zeros((NCORES, RB, P, 8 * H), np.int16)
    smat = np.zeros((NCORES, P, RB * T * BR), ml_dtypes.bfloat16)
    iota = np.arange(BR, dtype=np.float32)
    for h in range(NCORES):
        for blk in range(RB):
            rb, cb, vb, nlow = per_core[h][blk]
            nh = len(rb) - nlow
            lowidx[h, blk] = _pack_idx(cb[:nlow], L * P)
            highidx[h, blk] = _pack_idx(cb[nlow:] - SPLIT, H * P)
            rr = np.zeros(T * P, np.float32)
            vv = np.zeros(T * P, np.float32)
            rr[:nlow] = (rb[:nlow] - blk * BR).astype(np.float32)
            vv[:nlow] = vb[:nlow]
            rr[L * P:L * P + nh] = (rb[nlow:] - blk * BR).astype(np.float32)
            vv[L * P:L * P + nh] = vb[nlow:]
            # S[e, r] for slot e=t*P+p -> smat[p, (blk*T+t)*BR + r]
            s_f32 = (iota[None, :] == rr[:, None]) * vv[:, None]  # [T*P, BR]
            smat[h, :, blk * T * BR:(blk + 1) * T * BR] = (
                s_f32.reshape(T, P, BR).transpose(1, 0, 2).reshape(P, T * BR)
                .astype(ml_dtypes.bfloat16))
    return lowidx, highidx, smat, L, H


def _build_program(L, H, n_blocks=RB, n_rows=N):
    import concourse.bacc as bacc
    import concourse.tile as tile
    from concourse import mybir
    from concourse._compat import get_trn_type

    T = L + H
    BC = B * C                       # 512 feature cols in xcat
    f32 = mybir.dt.float32
    bf16 = mybir.dt.bfloat16
    i16 = mybir.dt.int16
    nc = bacc.Bacc(get_trn_type() or "TRN2", target_bir_lowering=False)

    x_d = nc.dram_tensor("xcat", [n_rows, BC], bf16, kind="ExternalInput")
    lowidx_d = nc.dram_tensor("lowidx", [P, n_blocks * 8 * L], i16,
                              kind="ExternalInput")
    highidx_d = nc.dram_tensor("highidx", [P, n_blocks * 8 * H], i16,
                               kind="ExternalInput")
    smat_d = nc.dram_tensor("smat", [P, n_blocks * T * BR], bf16,
                            kind="ExternalInput")
    wt_d = nc.dram_tensor("wt", [C, C], bf16, kind="ExternalInput")
    bias_d = nc.dram_tensor("bias", [C, 1], f32, kind="ExternalInput")
    out_d = nc.dram_tensor("outT", [B, C, n_blocks * BR], f32,
                           kind="ExternalOutput")

    with tile.TileContext(nc) as tc:
        with (
            tc.tile_pool(name="const", bufs=1) as const_pool,
            tc.tile_pool(name="meta", bufs=1) as meta_pool,
            tc.tile_pool(name="msgs", bufs=3) as msgs_pool,
            tc.tile_pool(name="smat", bufs=3) as s_pool,
            tc.tile_pool(name="aggsb", bufs=2) as agg_pool,
            tc.tile_pool(name="ostage", bufs=2) as ostage_pool,
            tc.tile_pool(name="psum_agg", bufs=2, space="PSUM") as psA,
            tc.tile_pool(name="psum_out", bufs=2, space="PSUM") as psO,
        ):
            wt_sb = const_pool.tile([C, C], bf16)
            bias_sb = const_pool.tile([C, 1], f32)
            nc.sync.dma_start(out=wt_sb[:], in_=wt_d[:])
            nc.sync.dma_start(out=bias_sb[:], in_=bias_d[:])

            lowidx_sb = meta_pool.tile([P, n_blocks * 8 * L], i16)
            highidx_sb = meta_pool.tile([P, n_blocks * 8 * H], i16)
            nc.sync.dma_start(out=lowidx_sb[:], in_=lowidx_d[:])
            nc.sync.dma_start(out=highidx_sb[:], in_=highidx_d[:])

            ostage = None
            for blk in range(n_blocks):
                msgs = msgs_pool.tile([P, T, BC], bf16)
                nc.gpsimd.dma_gather(
                    out_ap=msgs[:, :L, :],
                    in_ap=x_d[:SPLIT, :],
                    idxs_ap=lowidx_sb[:, blk * 8 * L:(blk + 1) * 8 * L],
                    num_idxs=L * P,
                    num_idxs_reg=L * P,
                    elem_size=BC,
                    single_packet=False,
                )
                nc.gpsimd.dma_gather(
                    out_ap=msgs[:, L:, :],
                    in_ap=x_d[SPLIT:, :],
                    idxs_ap=highidx_sb[:, blk * 8 * H:(blk + 1) * 8 * H],
                    num_idxs=H * P,
                    num_idxs_reg=H * P,
                    elem_size=BC,
                    single_packet=False,
                )
                s_blk = s_pool.tile([P, T * BR], bf16)
                nc.sync.dma_start(
                    out=s_blk[:],
                    in_=smat_d[:, blk * T * BR:(blk + 1) * T * BR])
                aggT_ps = psA.tile([C, B * BR], f32)
                for bb in range(B):
                    for t in range(T):
                        nc.tensor.matmul(
                            out=aggT_ps[:, bb * BR:(bb + 1) * BR],
                            lhsT=msgs[:, t, bb * C:(bb + 1) * C],
                            rhs=s_blk[:, t * BR:(t + 1) * BR],
                            start=(t == 0), stop=(t == T - 1),
                        )
                aggT_sb = agg_pool.tile([C, B * BR], bf16)
                nc.vector.tensor_copy(out=aggT_sb[:], in_=aggT_ps[:])
                outT_ps = psO.tile([C, B * BR], f32)
                for bb in range(B):
                    nc.tensor.matmul(
                        out=outT_ps[:, bb * BR:(bb + 1) * BR],
                        lhsT=wt_sb[:],
                        rhs=aggT_sb[:, bb * BR:(bb + 1) * BR],
                        start=True, stop=True)
                if blk % OUT_DMA_BLKS == 0:
                    ostage = ostage_pool.tile([C, B, OUT_DMA_BLKS * BR], f32)
                o_off = (blk % OUT_DMA_BLKS) * BR
                for bb in range(B):
                    nc.scalar.activation(
                        out=ostage[:, bb, o_off:o_off + BR],
                        in_=outT_ps[:, bb * BR:(bb + 1) * BR],
                        func=mybir.ActivationFunctionType.Relu,
                        bias=bias_sb[:, :1], scale=1.0,
                    )
                if blk % OUT_DMA_BLKS == OUT_DMA_BLKS - 1 or blk == n_blocks - 1:
                    lo_blk = (blk // OUT_DMA_BLKS) * OUT_DMA_BLKS
                    width = (blk - lo_blk + 1) * BR
                    for bb in range(B):
                        nc.sync.dma_start(
                            out=out_d[bb, :, lo_blk * BR: lo_blk * BR + width],
                            in_=ostage[:, bb, :width],
                        )
    return nc


def _ensure_ntff_hook_importable():
    """bass_utils imports antenv.axon_hooks when BASS_TRACE is set; this
    image lacks that module. Provide a null hook so tracing degrades
    gracefully instead of crashing."""
    import types

    try:
        import antenv.axon_hooks  # noqa: F401
        return
    except ImportError:
        pass
    mod = types.ModuleType("antenv.axon_hooks")
    mod.get_axon_ntff_profile_hook = lambda: None
    mod.set_axon_ntff_profile_hook = lambda h: None
    sys.modules["antenv.axon_hooks"] = mod
    try:
        import antenv
        antenv.axon_hooks = mod
    except ImportError:
        pass


def kernel(x, edge_row, edge_col, edge_vals, W, b):
    _ensure_ntff_hook_importable()
    from concourse.bass_utils import run_bass_kernel_spmd

    x = np.asarray(x, np.float32)
    edge_row = np.asarray(edge_row, np.int32)
    edge_col = np.asarray(edge_col, np.int32)
    edge_vals = np.asarray(edge_vals, np.float32)
    W = np.asarray(W, np.float32)
    b = np.asarray(b, np.float32)

    lowidx, highidx, smat, L, H = _preprocess(edge_row, edge_col, edge_vals)
    nc = _build_program(L, H)
    nc.compile()

    # xcat[n] = x[:, n, :] flattened -> [N, 4*128] bf16
    xcat = np.ascontiguousarray(
        x.transpose(1, 0, 2).reshape(N, B * C)).astype(ml_dtypes.bfloat16)
    wt = W.astype(ml_dtypes.bfloat16)
    in_maps = []
    for h in range(NCORES):
        in_maps.append({
            "xcat": xcat,
            "lowidx": np.ascontiguousarray(
                lowidx[h].transpose(1, 0, 2).reshape(P, RB * 8 * L)),
            "highidx": np.ascontiguousarray(
                highidx[h].transpose(1, 0, 2).reshape(P, RB * 8 * H)),
            "smat": smat[h],
            "wt": wt,
            "bias": np.ascontiguousarray(b[:, None]),
        })

    res = run_bass_kernel_spmd(nc, in_maps, list(range(NCORES)))
    global LAST_RESULTS
    LAST_RESULTS = res

    out = np.empty((B, N, C), np.float32)
    for h in range(NCORES):
        lo, hi = h * RH, min((h + 1) * RH, N)
        o = res.results[h]["outT"]              # [B, C, RB*BR]
        for bb in range(B):
            out[bb, lo:hi] = o[bb].T[:hi - lo]
    return out



# revision 2
# speedup vs baseline: 1.0538x; 1.0538x over previous
"""Trainium2 Bass kernel for GCNN message passing.

out[b] = relu((A @ x[b]) @ W + bias),  A sparse [N, N] from 800k edges.

Sharding (8 NeuronCores): destination rows are partitioned into 200 blocks
of 250 rows each (25 blocks per core), load-balanced so every block has
nearly the same number of low-column (col < 32768) and high-column edges.
Host interleaves x into xcat[n] = x[:, n, :] (bf16, [N, 4*128]) so ONE
gather descriptor fetches a neighbor's features for all 4 batches at once.

The SWDGE descriptor-generation on the Pool engine is the bottleneck
resource (~8ns per gather index on one queue).  Two SWDGE queues are used
with alternating gathers, which pipelines descriptor generation across two
Q7 cpu pairs (~4.4ns/idx measured).

Device algorithm per core, per row-block (256-row span, 250 used):
  - two dma_gather ops (low cols on queue 0, high cols on queue 1) fetch
    msgs [128(edge), T, 512] bf16; edge slot k -> partition k%128,
    tile k//128.
  - the scaled one-hot scatter matrices S[slot, r] = (r == row[slot]) *
    val[slot] are built ON DEVICE by the (otherwise idle) Vector engine:
    S_tile = (iota == rows_scalar) * vals_scalar  via tensor_scalar,
    from compact per-slot row/val tables ([128, 25*T] f32 each).
  - PE accumulates aggT_b[c, r] += msgs[:, t, b*128:+128].T @ S_t into
    PSUM [128, 4*256] f32 (segment sum via matmul accumulation).
  - aggT -> SBUF bf16, PE applies W (outT_b = W.T @ aggT_b) into a
    second PSUM tile, ACT applies relu(.+bias), batched DMA writes
    outT [4, 128, 6400] f32.
Host scatters the per-block columns back to original row order.
"""
import sys

import numpy as np

try:  # concourse (Bass) lives in the trn repo
    import concourse  # noqa: F401
except ImportError:  # pragma: no cover
    sys.path.insert(0, "/opt/trn_rl_repo")

import ml_dtypes

B, N, E, C = 4, 50000, 800000, 128
LAST_RESULTS = None  # BassKernelResults of the most recent kernel() call
P = 128
BR = 256            # row span of a block's PSUM tile (250 rows used)
RB = 25             # row-blocks per core
NBLK = 200          # total row-blocks (8 cores x 25)
RPB = 250           # rows assigned per block (200 * 250 = 50000)
NCORES = 8
SPLIT = 32768       # low/high column split for int16 gather indices
OUT_DMA_BLKS = 4    # row-blocks per output DMA


def _balance_rows(nlo, nhi):
    """Assign each of the N rows to one of NBLK blocks (RPB rows each) so
    per-block low/high edge counts are near-uniform.

    Chunked alternating-key matching: rows sorted by total degree are
    processed in chunks of NBLK; each chunk assigns one row per block,
    pairing heavy rows with light blocks (alternating the balancing key
    between low and high counts).  Returns assignment [N] -> block id.
    """
    order = np.argsort(-(nlo + nhi), kind="stable")
    blk_lo = np.zeros(NBLK, np.int64)
    blk_hi = np.zeros(NBLK, np.int64)
    assignment = np.empty(N, np.int32)
    for i in range(RPB):
        idx = order[i * NBLK:(i + 1) * NBLK]
        if i % 2 == 0:
            rsort = idx[np.argsort(-nlo[idx], kind="stable")]
            bsort = np.argsort(blk_lo, kind="stable")
        else:
            rsort = idx[np.argsort(-nhi[idx], kind="stable")]
            bsort = np.argsort(blk_hi, kind="stable")
        assignment[rsort] = bsort
        blk_lo[bsort] += nlo[rsort]
        blk_hi[bsort] += nhi[rsort]
    return assignment, int(blk_lo.max()), int(blk_hi.max())


def _pack_idx_blocks(vals_per_block, n_slots):
    """Pack per-block int16 index vectors [nblk, n_slots] into the SWDGE
    layout: index k at [k % 16, k // 16], replicated to 128 partitions.
    Returns [128, nblk * (n_slots // 16)]."""
    nblk = vals_per_block.shape[0]
    t16 = vals_per_block.reshape(nblk, n_slots // 16, 16).transpose(0, 2, 1)
    t128 = np.tile(t16, (1, 8, 1))              # [nblk, 128, n_slots//16]
    return np.ascontiguousarray(
        t128.transpose(1, 0, 2).reshape(P, nblk * (n_slots // 16)))


def _preprocess(edge_row, edge_col, edge_vals):
    """Balanced block partition + per-core gather index tables and compact
    per-slot (row, val) tables for the on-device S build."""
    is_hi = edge_col >= SPLIT
    nlo = np.bincount(edge_row[~is_hi], minlength=N)
    nhi = np.bincount(edge_row[is_hi], minlength=N)
    assignment, max_lo, max_hi = _balance_rows(nlo, nhi)

    L = (max_lo + P - 1) // P
    H = (max_hi + P - 1) // P
    T = L + H

    # local row index within block: position in the block's row list
    perm = np.argsort(assignment, kind="stable")     # rows grouped by block
    block_rows = perm.reshape(NBLK, RPB)             # [block, local] -> row
    rloc = np.empty(N, np.int32)
    rloc[perm] = np.tile(np.arange(RPB, dtype=np.int32), NBLK)

    eblk = assignment[edge_row]                      # block of each edge
    erloc = rloc[edge_row]
    order = np.lexsort((edge_col, is_hi, eblk))
    eb, ehi, ec, er, ev = (eblk[order], is_hi[order], edge_col[order],
                           erloc[order], edge_vals[order])

    # slot within (block, group): cumcount via group-start offsets
    gkey = eb.astype(np.int64) * 2 + ehi
    starts = np.searchsorted(gkey, np.arange(NBLK * 2))
    slot = np.arange(E) - starts[gkey]
    gslot = np.where(ehi, L * P + slot, slot)        # slot within block
    tile = gslot // P
    part = gslot % P

    lowidx = np.zeros((NBLK, L * P), np.int16)
    highidx = np.zeros((NBLK, H * P), np.int16)
    lowidx[eb[~ehi], slot[~ehi]] = ec[~ehi].astype(np.int16)
    highidx[eb[ehi], slot[ehi]] = (ec[ehi] - SPLIT).astype(np.int16)

    rowsv = np.zeros((NBLK, P, T), np.float32)
    valsv = np.zeros((NBLK, P, T), np.float32)
    rowsv[eb, part, tile] = er.astype(np.float32)
    valsv[eb, part, tile] = ev

    per_core = []
    for h in range(NCORES):
        s = slice(h * RB, (h + 1) * RB)
        per_core.append({
            "lowidx": _pack_idx_blocks(lowidx[s], L * P),
            "highidx": _pack_idx_blocks(highidx[s], H * P),
            "rowsv": np.ascontiguousarray(
                rowsv[s].transpose(1, 0, 2).reshape(P, RB * T)),
            "valsv": np.ascontiguousarray(
                valsv[s].transpose(1, 0, 2).reshape(P, RB * T)),
        })
    return per_core, block_rows, L, H


def _build_program(L, H):
    import concourse.bacc as bacc
    import concourse.tile as tile
    from concourse import mybir
    from concourse._compat import get_trn_type

    T = L + H
    BC = B * C                       # 512 feature cols in xcat
    f32 = mybir.dt.float32
    bf16 = mybir.dt.bfloat16
    i16 = mybir.dt.int16
    nc = bacc.Bacc(get_trn_type() or "TRN2", target_bir_lowering=False,
                   num_swdge_queues=2)

    x_d = nc.dram_tensor("xcat", [N, BC], bf16, kind="ExternalInput")
    lowidx_d = nc.dram_tensor("lowidx", [P, RB * 8 * L], i16,
                              kind="ExternalInput")
    highidx_d = nc.dram_tensor("highidx", [P, RB * 8 * H], i16,
                               kind="ExternalInput")
    rowsv_d = nc.dram_tensor("rowsv", [P, RB * T], f32, kind="ExternalInput")
    valsv_d = nc.dram_tensor("valsv", [P, RB * T], f32, kind="ExternalInput")
    iota_d = nc.dram_tensor("iota", [P, BR], f32, kind="ExternalInput")
    wt_d = nc.dram_tensor("wt", [C, C], bf16, kind="ExternalInput")
    bias_d = nc.dram_tensor("bias", [C, 1], f32, kind="ExternalInput")
    out_d = nc.dram_tensor("outT", [B, C, RB * BR], f32,
                           kind="ExternalOutput")

    with tile.TileContext(nc) as tc:
        with (
            tc.tile_pool(name="const", bufs=1) as const_pool,
            tc.tile_pool(name="meta", bufs=1) as meta_pool,
            tc.tile_pool(name="msgs", bufs=3) as msgs_pool,
            tc.tile_pool(name="smat", bufs=2) as s_pool,
            tc.tile_pool(name="aggsb", bufs=2) as agg_pool,
            tc.tile_pool(name="ostage", bufs=2) as ostage_pool,
            tc.tile_pool(name="psum_agg", bufs=2, space="PSUM") as psA,
            tc.tile_pool(name="psum_out", bufs=2, space="PSUM") as psO,
        ):
            wt_sb = const_pool.tile([C, C], bf16)
            bias_sb = const_pool.tile([C, 1], f32)
            iota_sb = const_pool.tile([P, BR], f32)
            nc.sync.dma_start(out=wt_sb[:], in_=wt_d[:])
            nc.sync.dma_start(out=bias_sb[:], in_=bias_d[:])
            nc.sync.dma_start(out=iota_sb[:], in_=iota_d[:])

            lowidx_sb = meta_pool.tile([P, RB * 8 * L], i16)
            highidx_sb = meta_pool.tile([P, RB * 8 * H], i16)
            rowsv_sb = meta_pool.tile([P, RB * T], f32)
            valsv_sb = meta_pool.tile([P, RB * T], f32)
            nc.sync.dma_start(out=lowidx_sb[:], in_=lowidx_d[:])
            nc.sync.dma_start(out=highidx_sb[:], in_=highidx_d[:])
            nc.sync.dma_start(out=rowsv_sb[:], in_=rowsv_d[:])
            nc.sync.dma_start(out=valsv_sb[:], in_=valsv_d[:])

            ostage = None
            for blk in range(RB):
                msgs = msgs_pool.tile([P, T, BC], bf16)
                nc.gpsimd.dma_gather(
                    out_ap=msgs[:, :L, :],
                    in_ap=x_d[:SPLIT, :],
                    idxs_ap=lowidx_sb[:, blk * 8 * L:(blk + 1) * 8 * L],
                    num_idxs=L * P,
                    num_idxs_reg=L * P,
                    elem_size=BC,
                    single_packet=False,
                    queue_num=0,
                )
                nc.gpsimd.dma_gather(
                    out_ap=msgs[:, L:, :],
                    in_ap=x_d[SPLIT:, :],
                    idxs_ap=highidx_sb[:, blk * 8 * H:(blk + 1) * 8 * H],
                    num_idxs=H * P,
                    num_idxs_reg=H * P,
                    elem_size=BC,
                    single_packet=False,
                    queue_num=1,
                )
                s_blk = s_pool.tile([P, T * BR], bf16)
                for t in range(T):
                    nc.vector.tensor_scalar(
                        out=s_blk[:, t * BR:(t + 1) * BR],
                        in0=iota_sb[:],
                        scalar1=rowsv_sb[:, blk * T + t:blk * T + t + 1],
                        scalar2=valsv_sb[:, blk * T + t:blk * T + t + 1],
                        op0=mybir.AluOpType.is_equal,
                        op1=mybir.AluOpType.mult,
                    )
                aggT_ps = psA.tile([C, B * BR], f32)
                for bb in range(B):
                    for t in range(T):
                        nc.tensor.matmul(
                            out=aggT_ps[:, bb * BR:(bb + 1) * BR],
                            lhsT=msgs[:, t, bb * C:(bb + 1) * C],
                            rhs=s_blk[:, t * BR:(t + 1) * BR],
                            start=(t == 0), stop=(t == T - 1),
                        )
                aggT_sb = agg_pool.tile([C, B * BR], bf16)
                nc.vector.tensor_copy(out=aggT_sb[:], in_=aggT_ps[:])
                outT_ps = psO.tile([C, B * BR], f32)
                for bb in range(B):
                    nc.tensor.matmul(
                        out=outT_ps[:, bb * BR:(bb + 1) * BR],
                        lhsT=wt_sb[:],
                        rhs=aggT_sb[:, bb * BR:(bb + 1) * BR],
                        start=True, stop=True)
                if blk % OUT_DMA_BLKS == 0:
                    ostage = ostage_pool.tile([C, B, OUT_DMA_BLKS * BR], f32)
                o_off = (blk % OUT_DMA_BLKS) * BR
                for bb in range(B):
                    nc.scalar.activation(
                        out=ostage[:, bb, o_off:o_off + BR],
                        in_=outT_ps[:, bb * BR:(bb + 1) * BR],
                        func=mybir.ActivationFunctionType.Relu,
                        bias=bias_sb[:, :1], scale=1.0,
                    )
                if blk % OUT_DMA_BLKS == OUT_DMA_BLKS - 1 or blk == RB - 1:
                    lo_blk = (blk // OUT_DMA_BLKS) * OUT_DMA_BLKS
                    width = (blk - lo_blk + 1) * BR
                    for bb in range(B):
                        nc.sync.dma_start(
                            out=out_d[bb, :, lo_blk * BR: lo_blk * BR + width],
                            in_=ostage[:, bb, :width],
                        )
    return nc


def _ensure_ntff_hook_importable():
    """bass_utils imports antenv.axon_hooks when BASS_TRACE is set; this
    image lacks that module. Provide a null hook so tracing degrades
    gracefully instead of crashing."""
    import types

    try:
        import antenv.axon_hooks  # noqa: F401
        return
    except ImportError:
        pass
    mod = types.ModuleType("antenv.axon_hooks")
    mod.get_axon_ntff_profile_hook = lambda: None
    mod.set_axon_ntff_profile_hook = lambda h: None
    sys.modules["antenv.axon_hooks"] = mod
    try:
        import antenv
        antenv.axon_hooks = mod
    except ImportError:
        pass


def kernel(x, edge_row, edge_col, edge_vals, W, b):
    _ensure_ntff_hook_importable()
    from concourse.bass_utils import run_bass_kernel_spmd

    x = np.asarray(x, np.float32)
    edge_row = np.asarray(edge_row, np.int32)
    edge_col = np.asarray(edge_col, np.int32)
    edge_vals = np.asarray(edge_vals, np.float32)
    W = np.asarray(W, np.float32)
    b = np.asarray(b, np.float32)

    per_core, block_rows, L, H = _preprocess(edge_row, edge_col, edge_vals)
    nc = _build_program(L, H)
    nc.compile()

    # xcat[n] = x[:, n, :] flattened -> [N, 4*128] bf16
    xcat = np.ascontiguousarray(
        x.transpose(1, 0, 2).reshape(N, B * C)).astype(ml_dtypes.bfloat16)
    wt = W.astype(ml_dtypes.bfloat16)
    iota = np.broadcast_to(
        np.arange(BR, dtype=np.float32)[None, :], (P, BR)).copy()
    in_maps = []
    for h in range(NCORES):
        in_maps.append({
            "xcat": xcat,
            "lowidx": per_core[h]["lowidx"],
            "highidx": per_core[h]["highidx"],
            "rowsv": per_core[h]["rowsv"],
            "valsv": per_core[h]["valsv"],
            "iota": iota,
            "wt": wt,
            "bias": np.ascontiguousarray(b[:, None]),
        })

    res = run_bass_kernel_spmd(nc, in_maps, list(range(NCORES)))
    global LAST_RESULTS
    LAST_RESULTS = res

    # columns bb*BR + i (i < RPB) of core h hold row block_rows[h*RB+bb][i]
    pos = (np.arange(RB)[:, None] * BR + np.arange(RPB)[None, :]).ravel()
    out = np.empty((B, N, C), np.float32)
    for h in range(NCORES):
        o = res.results[h]["outT"]              # [B, C, RB*BR]
        rows = block_rows[h * RB:(h + 1) * RB].ravel()
        out[:, rows, :] = o[:, :, pos].transpose(0, 2, 1)
    return out


# revision 10
# speedup vs baseline: 1.4872x; 1.4112x over previous
"""Trainium2 Bass kernel for GCNN message passing.

out[b] = relu((A @ x[b]) @ W + bias),  A sparse [N, N] from 800k edges.

Sharding (8 NeuronCores): destination rows are partitioned into 200 blocks
of 250 rows each (25 blocks per core), load-balanced so every block has
nearly the same number of low-column (col < 32768) and high-column edges.
Host interleaves x into xcat[n] = x[:, n, :] (bf16, [N, 4*128]) so ONE
gather descriptor fetches a neighbor's features for all 4 batches at once.

The SWDGE descriptor-generation on the Pool engine is the bottleneck
resource (~8ns per gather index on one queue).  Two SWDGE queues are used
with alternating gathers, which pipelines descriptor generation across two
Q7 cpu pairs (~4.4ns/idx measured).

Device algorithm per core, per row-block (256-row span, 250 used):
  - two dma_gather ops (low cols on queue 0, high cols on queue 1) fetch
    msgs [128(edge), T, 512] bf16; edge slot k -> partition k%128,
    tile k//128.
  - the scaled one-hot scatter matrices S[slot, r] = (r == row[slot]) *
    val[slot] are built ON DEVICE by the (otherwise idle) Vector engine:
    S_tile = (iota == rows_scalar) * vals_scalar  via tensor_scalar,
    from compact per-slot row/val tables ([128, 25*T] f32 each).
  - PE accumulates aggT_b[c, r] += msgs[:, t, b*128:+128].T @ S_t into
    PSUM [128, 4*256] f32 (segment sum via matmul accumulation).
  - aggT -> SBUF bf16, PE applies W (outT_b = W.T @ aggT_b) into a
    second PSUM tile, ACT applies relu(.+bias), batched DMA writes
    outT [4, 128, 6400] f32.
Host scatters the per-block columns back to original row order.
"""
import sys

import numpy as np

try:  # concourse (Bass) lives in the trn repo
    import concourse  # noqa: F401
except ImportError:  # pragma: no cover
    sys.path.insert(0, "/opt/trn_rl_repo")

import ml_dtypes

B, N, E, C = 4, 50000, 800000, 128
LAST_RESULTS = None  # BassKernelResults of the most recent kernel() call
P = 128
BR = 256            # row span of a block's PSUM tile (250 rows used)
RB = 25             # row-blocks per core
NBLK = 200          # total row-blocks (8 cores x 25)
RPB = 250           # rows assigned per block (200 * 250 = 50000)
NCORES = 8
SPLIT = 32768       # low/high column split for int16 gather indices
OUT_DMA_BLKS = 4    # row-blocks per output DMA


def _balance_rows(nlo, nhi):
    """Assign each of the N rows to one of NBLK blocks (RPB rows each) so
    per-block low/high edge counts are near-uniform.

    Chunked alternating-key matching: rows sorted by total degree are
    processed in chunks of NBLK; each chunk assigns one row per block,
    pairing heavy rows with light blocks (alternating the balancing key
    between low and high counts).  Returns assignment [N] -> block id.
    """
    order = np.argsort(-(nlo + nhi), kind="stable")
    blk_lo = np.zeros(NBLK, np.int64)
    blk_hi = np.zeros(NBLK, np.int64)
    assignment = np.empty(N, np.int32)
    for i in range(RPB):
        idx = order[i * NBLK:(i + 1) * NBLK]
        if i % 2 == 0:
            rsort = idx[np.argsort(-nlo[idx], kind="stable")]
            bsort = np.argsort(blk_lo, kind="stable")
        else:
            rsort = idx[np.argsort(-nhi[idx], kind="stable")]
            bsort = np.argsort(blk_hi, kind="stable")
        assignment[rsort] = bsort
        blk_lo[bsort] += nlo[rsort]
        blk_hi[bsort] += nhi[rsort]
    return assignment, int(blk_lo.max()), int(blk_hi.max())


def _pack_idx_blocks(vals_per_block, n_slots):
    """Pack per-block int16 index vectors [nblk, n_slots] into the SWDGE
    layout: index k at [k % 16, k // 16], replicated to 128 partitions.
    Returns [128, nblk * (n_slots // 16)]."""
    nblk = vals_per_block.shape[0]
    t16 = vals_per_block.reshape(nblk, n_slots // 16, 16).transpose(0, 2, 1)
    t128 = np.tile(t16, (1, 8, 1))              # [nblk, 128, n_slots//16]
    return np.ascontiguousarray(
        t128.transpose(1, 0, 2).reshape(P, nblk * (n_slots // 16)))


def _preprocess(edge_row, edge_col, edge_vals):
    """Balanced block partition + per-core gather index tables and compact
    per-slot (row, val) tables for the on-device S build."""
    is_hi = edge_col >= SPLIT
    nlo = np.bincount(edge_row[~is_hi], minlength=N)
    nhi = np.bincount(edge_row[is_hi], minlength=N)
    assignment, max_lo, max_hi = _balance_rows(nlo, nhi)

    L = (max_lo + P - 1) // P
    H = (max_hi + P - 1) // P
    T = L + H

    # local row index within block: position in the block's row list
    perm = np.argsort(assignment, kind="stable")     # rows grouped by block
    block_rows = perm.reshape(NBLK, RPB)             # [block, local] -> row
    rloc = np.empty(N, np.int32)
    rloc[perm] = np.tile(np.arange(RPB, dtype=np.int32), NBLK)

    eblk = assignment[edge_row]                      # block of each edge
    erloc = rloc[edge_row]
    order = np.lexsort((edge_col, is_hi, eblk))
    eb, ehi, ec, er, ev = (eblk[order], is_hi[order], edge_col[order],
                           erloc[order], edge_vals[order])

    # slot within (block, group): cumcount via group-start offsets
    gkey = eb.astype(np.int64) * 2 + ehi
    starts = np.searchsorted(gkey, np.arange(NBLK * 2))
    slot = np.arange(E) - starts[gkey]
    gslot = np.where(ehi, L * P + slot, slot)        # slot within block
    tile = gslot // P
    part = gslot % P

    lowidx = np.zeros((NBLK, L * P), np.int16)
    highidx = np.zeros((NBLK, H * P), np.int16)
    lowidx[eb[~ehi], slot[~ehi]] = ec[~ehi].astype(np.int16)
    highidx[eb[ehi], slot[ehi]] = (ec[ehi] - SPLIT).astype(np.int16)

    rowsv = np.zeros((NBLK, P, T), ml_dtypes.bfloat16)
    valsv = np.zeros((NBLK, P, T), ml_dtypes.bfloat16)
    rowsv[eb, part, tile] = er.astype(ml_dtypes.bfloat16)
    valsv[eb, part, tile] = ev.astype(ml_dtypes.bfloat16)

    per_core = []
    for h in range(NCORES):
        s = slice(h * RB, (h + 1) * RB)
        per_core.append({
            "lowidx": _pack_idx_blocks(lowidx[s], L * P),
            "highidx": _pack_idx_blocks(highidx[s], H * P),
            "rowsv": np.ascontiguousarray(
                rowsv[s].transpose(1, 0, 2).reshape(P, RB * T)),
            "valsv": np.ascontiguousarray(
                valsv[s].transpose(1, 0, 2).reshape(P, RB * T)),
        })
    return per_core, block_rows, L, H


def _build_program(L, H):
    import concourse.bacc as bacc
    import concourse.tile as tile
    from concourse import mybir
    from concourse._compat import get_trn_type

    T = L + H
    BC = B * C                       # 512 feature cols in xcat
    f32 = mybir.dt.float32
    bf16 = mybir.dt.bfloat16
    i16 = mybir.dt.int16
    nc = bacc.Bacc(get_trn_type() or "TRN2", target_bir_lowering=False,
                   num_swdge_queues=2)

    x_d = nc.dram_tensor("xcat", [N, BC], bf16, kind="ExternalInput")
    lowidx_d = nc.dram_tensor("lowidx", [P, RB * 8 * L], i16,
                              kind="ExternalInput")
    highidx_d = nc.dram_tensor("highidx", [P, RB * 8 * H], i16,
                               kind="ExternalInput")
    rowsv_d = nc.dram_tensor("rowsv", [P, RB * T], bf16, kind="ExternalInput")
    valsv_d = nc.dram_tensor("valsv", [P, RB * T], bf16, kind="ExternalInput")
    iota_d = nc.dram_tensor("iota", [P, BR], bf16, kind="ExternalInput")
    wt_d = nc.dram_tensor("wt", [C, C], bf16, kind="ExternalInput")
    bias_d = nc.dram_tensor("bias", [C, 1], f32, kind="ExternalInput")
    out_d = nc.dram_tensor("outT", [B, C, RB * BR], f32,
                           kind="ExternalOutput")

    with tile.TileContext(nc) as tc:
        with (
            tc.tile_pool(name="const", bufs=1) as const_pool,
            tc.tile_pool(name="meta", bufs=1) as meta_pool,
            tc.tile_pool(name="msgs", bufs=3) as msgs_pool,
            tc.tile_pool(name="smat", bufs=2) as s_pool,
            tc.tile_pool(name="aggsb", bufs=2) as agg_pool,
            tc.tile_pool(name="ostage", bufs=2) as ostage_pool,
            tc.tile_pool(name="psum_agg", bufs=2, space="PSUM") as psA,
            tc.tile_pool(name="psum_out", bufs=2, space="PSUM") as psO,
        ):
            wt_sb = const_pool.tile([C, C], bf16)
            bias_sb = const_pool.tile([C, 1], f32)
            iota_sb = const_pool.tile([P, BR], bf16)
            nc.sync.dma_start(out=wt_sb[:], in_=wt_d[:])
            nc.sync.dma_start(out=bias_sb[:], in_=bias_d[:])
            nc.sync.dma_start(out=iota_sb[:], in_=iota_d[:])

            lowidx_sb = meta_pool.tile([P, RB * 8 * L], i16)
            highidx_sb = meta_pool.tile([P, RB * 8 * H], i16)
            rowsv_sb = meta_pool.tile([P, RB * T], bf16)
            valsv_sb = meta_pool.tile([P, RB * T], bf16)
            nc.sync.dma_start(out=lowidx_sb[:], in_=lowidx_d[:])
            nc.sync.dma_start(out=highidx_sb[:], in_=highidx_d[:])
            nc.sync.dma_start(out=rowsv_sb[:], in_=rowsv_d[:])
            nc.sync.dma_start(out=valsv_sb[:], in_=valsv_d[:])

            # split each (lo, hi) gather pair in two so the two SWDGE
            # queues carry equal descriptor-generation load per block
            La, Lb = (L + 1) // 2, L // 2
            Ha, Hb = (H + 1) // 2, H // 2
            ostage = None
            for blk in range(RB):
                msgs = msgs_pool.tile([P, T, BC], bf16)
                lo0 = blk * 8 * L
                hi0 = blk * 8 * H
                parts = [
                    (msgs[:, :La, :], x_d[:SPLIT, :],
                     lowidx_sb[:, lo0:lo0 + 8 * La], La, 0),
                    (msgs[:, La:L, :], x_d[:SPLIT, :],
                     lowidx_sb[:, lo0 + 8 * La:lo0 + 8 * L], Lb, 1),
                    (msgs[:, L:L + Ha, :], x_d[SPLIT:, :],
                     highidx_sb[:, hi0:hi0 + 8 * Ha], Ha, 1),
                    (msgs[:, L + Ha:, :], x_d[SPLIT:, :],
                     highidx_sb[:, hi0 + 8 * Ha:hi0 + 8 * H], Hb, 0),
                ]
                for out_ap, in_ap, idxs_ap, ntile, q in parts:
                    if ntile == 0:
                        continue
                    nc.gpsimd.dma_gather(
                        out_ap=out_ap,
                        in_ap=in_ap,
                        idxs_ap=idxs_ap,
                        num_idxs=ntile * P,
                        num_idxs_reg=ntile * P,
                        elem_size=BC,
                        single_packet=False,
                        queue_num=q,
                    )
                s_blk = s_pool.tile([P, T, BR], bf16)
                iota_brd = iota_sb[:].unsqueeze(1).broadcast_to([P, T, BR])
                rows_brd = (rowsv_sb[:, blk * T:(blk + 1) * T]
                            .unsqueeze(2).broadcast_to([P, T, BR]))
                vals_brd = (valsv_sb[:, blk * T:(blk + 1) * T]
                            .unsqueeze(2).broadcast_to([P, T, BR]))
                nc.vector.tensor_tensor(
                    out=s_blk[:], in0=iota_brd, in1=rows_brd,
                    op=mybir.AluOpType.is_equal)
                nc.vector.tensor_tensor(
                    out=s_blk[:], in0=s_blk[:], in1=vals_brd,
                    op=mybir.AluOpType.mult)
                aggT_ps = psA.tile([C, B * BR], f32)
                for bb in range(B):
                    for t in range(T):
                        nc.tensor.matmul(
                            out=aggT_ps[:, bb * BR:(bb + 1) * BR],
                            lhsT=msgs[:, t, bb * C:(bb + 1) * C],
                            rhs=s_blk[:, t, :],
                            start=(t == 0), stop=(t == T - 1),
                        )
                aggT_sb = agg_pool.tile([C, B * BR], bf16)
                nc.vector.tensor_copy(out=aggT_sb[:], in_=aggT_ps[:])
                outT_ps = psO.tile([C, B * BR], f32)
                for bb in range(B):
                    nc.tensor.matmul(
                        out=outT_ps[:, bb * BR:(bb + 1) * BR],
                        lhsT=wt_sb[:],
                        rhs=aggT_sb[:, bb * BR:(bb + 1) * BR],
                        start=True, stop=True)
                if blk % OUT_DMA_BLKS == 0:
                    ostage = ostage_pool.tile([C, B, OUT_DMA_BLKS * BR], f32)
                o_off = (blk % OUT_DMA_BLKS) * BR
                for bb in range(B):
                    nc.scalar.activation(
                        out=ostage[:, bb, o_off:o_off + BR],
                        in_=outT_ps[:, bb * BR:(bb + 1) * BR],
                        func=mybir.ActivationFunctionType.Relu,
                        bias=bias_sb[:, :1], scale=1.0,
                    )
                if blk % OUT_DMA_BLKS == OUT_DMA_BLKS - 1 or blk == RB - 1:
                    lo_blk = (blk // OUT_DMA_BLKS) * OUT_DMA_BLKS
                    width = (blk - lo_blk + 1) * BR
                    for bb in range(B):
                        nc.sync.dma_start(
                            out=out_d[bb, :, lo_blk * BR: lo_blk * BR + width],
                            in_=ostage[:, bb, :width],
                        )
    return nc


def _ensure_ntff_hook_importable():
    """bass_utils imports antenv.axon_hooks when BASS_TRACE is set; this
    image lacks that module. Provide a null hook so tracing degrades
    gracefully instead of crashing."""
    import types

    try:
        import antenv.axon_hooks  # noqa: F401
        return
    except ImportError:
        pass
    mod = types.ModuleType("antenv.axon_hooks")
    mod.get_axon_ntff_profile_hook = lambda: None
    mod.set_axon_ntff_profile_hook = lambda h: None
    sys.modules["antenv.axon_hooks"] = mod
    try:
        import antenv
        antenv.axon_hooks = mod
    except ImportError:
        pass


def kernel(x, edge_row, edge_col, edge_vals, W, b):
    _ensure_ntff_hook_importable()
    from concourse.bass_utils import run_bass_kernel_spmd

    x = np.asarray(x, np.float32)
    edge_row = np.asarray(edge_row, np.int32)
    edge_col = np.asarray(edge_col, np.int32)
    edge_vals = np.asarray(edge_vals, np.float32)
    W = np.asarray(W, np.float32)
    b = np.asarray(b, np.float32)

    per_core, block_rows, L, H = _preprocess(edge_row, edge_col, edge_vals)
    nc = _build_program(L, H)
    nc.compile()

    # xcat[n] = x[:, n, :] flattened -> [N, 4*128] bf16
    xcat = np.ascontiguousarray(
        x.transpose(1, 0, 2).reshape(N, B * C)).astype(ml_dtypes.bfloat16)
    wt = W.astype(ml_dtypes.bfloat16)
    iota = np.broadcast_to(
        np.arange(BR, dtype=np.float32)[None, :],
        (P, BR)).astype(ml_dtypes.bfloat16)
    in_maps = []
    for h in range(NCORES):
        in_maps.append({
            "xcat": xcat,
            "lowidx": per_core[h]["lowidx"],
            "highidx": per_core[h]["highidx"],
            "rowsv": per_core[h]["rowsv"],
            "valsv": per_core[h]["valsv"],
            "iota": iota,
            "wt": wt,
            "bias": np.ascontiguousarray(b[:, None]),
        })

    res = run_bass_kernel_spmd(nc, in_maps, list(range(NCORES)))
    global LAST_RESULTS
    LAST_RESULTS = res

    # columns bb*BR + i (i < RPB) of core h hold row block_rows[h*RB+bb][i]
    pos = (np.arange(RB)[:, None] * BR + np.arange(RPB)[None, :]).ravel()
    out = np.empty((B, N, C), np.float32)
    for h in range(NCORES):
        o = res.results[h]["outT"]              # [B, C, RB*BR]
        rows = block_rows[h * RB:(h + 1) * RB].ravel()
        out[:, rows, :] = o[:, :, pos].transpose(0, 2, 1)
    return out
